# revision 15
# baseline (speedup 1.0000x reference)
"""AdaptiveModulatedConv3d — 8-core TRN2 Bass kernel.

Problem (hardcoded): BS=8, C_IN=C_OUT=64, K=3, STYLE_DIM=512, BANK=4,
D=H=W=32, pad=1, stride=1, f32 in/out.

Sharding: pure data-parallel over batch — each of the 8 NeuronCores gets one
sample, builds its per-sample demodulated conv weights on-device, and runs
its own 3D conv. No collectives.

Per-core conv strategy: the 3x3x3 conv is decomposed into 27 shifted
matmuls (contraction over C_IN=64) accumulating into PSUM. The PE 128x128
array is quadrant-packed: row-groups 0/64 hold two copies of x (bf16), so
two offset-matmuls run concurrently; col-groups 0/64 compute the two
h-halves of one output d-plane in the same PSUM bank. Boundary kernel taps
use narrowed-N matmuls instead of padding, so every DMA is contiguous.
"""

import numpy as np

import concourse.bass as bass
import concourse.tile as tile
from concourse import bacc, mybir
from concourse import bass_utils

F32 = mybir.dt.float32
BF16 = mybir.dt.bfloat16

BS = 8
CI = 64
CO = 64
KK = 3
SD = 512
BANK = 4
D = H = W = 32
EPS = 1e-8
NCORES = 8
DCH = 4  # d-planes per input-convert chunk

_CACHE = {}


def _emit_weight_build(nc, tc, pools, aps):
    """Build WT[128, 27, 64] bf16: WT[ci(+64), kd*9+kh*3+kw, co] =
    demodulated per-sample weight, duplicated on upper 64 partitions."""
    singles = pools["singles"]
    wk, fw, fb, mwt, mb, bankt = (
        aps["wk"], aps["fw"], aps["fb"], aps["mwt"], aps["mb"], aps["bankt"])

    # SBUF copies of the small params
    wk_sb = singles.tile([128, BANK], F32)
    nc.sync.dma_start(out=wk_sb, in_=wk)
    fw_sb = singles.tile([128, BANK, BANK], F32)
    nc.sync.dma_start(out=fw_sb, in_=fw)
    fb_sb = singles.tile([1, BANK], F32)
    nc.sync.dma_start(out=fb_sb, in_=fb)
    mwt_sb = singles.tile([128, BANK, CI], F32)
    nc.sync.dma_start(out=mwt_sb, in_=mwt)
    mb_sb = singles.tile([CI, 1], F32)
    nc.sync.dma_start(out=mb_sb, in_=mb)
    bank_sb = singles.tile([CI, BANK, 27 * CO], F32)
    nc.sync.dma_start(out=bank_sb, in_=bankt)

    ones1 = singles.tile([1, 64], F32)
    nc.vector.memset(ones1, 1.0)
    ones64 = singles.tile([64, 1], F32)
    nc.vector.memset(ones64, 1.0)

    with tc.tile_pool(name="wpsum", bufs=2, space="PSUM") as wpsum:
        # ---- filter weights: logits = w @ filter_w.T + filter_b ----
        ps_l = wpsum.tile([1, BANK], F32, tag="wps")
        for c in range(4):
            nc.tensor.matmul(ps_l, lhsT=wk_sb[:, c:c + 1], rhs=fw_sb[:, c, :],
                             start=(c == 0), stop=(c == 3))
        logits = singles.tile([1, BANK], F32)
        nc.vector.tensor_add(logits, ps_l, fb_sb)
        # softmax over 4 entries (logits are O(1), exp is safe)
        esb = singles.tile([1, BANK], F32)
        nc.scalar.activation(esb, logits, mybir.ActivationFunctionType.Exp)
        ssum = singles.tile([1, 1], F32)
        nc.vector.reduce_sum(out=ssum, in_=esb, axis=mybir.AxisListType.X)
        rsum = singles.tile([1, 1], F32)
        nc.vector.reciprocal(rsum, ssum)
        fwt = singles.tile([1, BANK], F32)
        nc.vector.tensor_scalar_mul(fwt, esb, rsum[:, 0:1])

        # ---- mod = w @ mod_w.T + mod_b  -> [ci, 1] ----
        ps_m = wpsum.tile([CI, 1], F32, tag="wps")
        for c in range(4):
            nc.tensor.matmul(ps_m, lhsT=mwt_sb[:, c, :], rhs=wk_sb[:, c:c + 1],
                             start=(c == 0), stop=(c == 3))
        mod_sb = singles.tile([CI, 1], F32)
        nc.vector.tensor_add(mod_sb, ps_m, mb_sb)

        # ---- broadcast fwt across partitions: [64, 4] ----
        ps_fb = wpsum.tile([64, BANK], F32, tag="wps")
        nc.tensor.matmul(ps_fb, lhsT=ones1, rhs=fwt, start=True, stop=True)
        fwt_b = singles.tile([64, BANK], F32)
        nc.vector.tensor_copy(fwt_b, ps_fb)

        # ---- weighted bank mix: acc[ci, koff*64+co] ----
        acc = singles.tile([CI, 27 * CO], F32)
        nc.vector.tensor_scalar_mul(acc, bank_sb[:, 0, :], fwt_b[:, 0:1])
        for n in range(1, 4):
            nc.vector.scalar_tensor_tensor(
                out=acc, in0=bank_sb[:, n, :], scalar=fwt_b[:, n:n + 1],
                in1=acc, op0=mybir.AluOpType.mult, op1=mybir.AluOpType.add)

        # ---- style modulation over input channels ----
        nc.vector.tensor_scalar_mul(acc, acc, mod_sb[:, 0:1])

        # ---- demod: rsqrt(sum_{ci,koff} mw^2 per co + eps) ----
        sq = singles.tile([CI, 27 * CO], F32)
        nc.scalar.square(sq, acc)
        partial = singles.tile([1, 4, CO], F32)
        chunks = [(0, 7), (7, 7), (14, 7), (21, 6)]  # koff ranges
        for j, (k0, nk) in enumerate(chunks):
            ps_c = wpsum.tile([1, nk * CO], F32, tag="wps")
            nc.tensor.matmul(ps_c, lhsT=ones64,
                             rhs=sq[:, k0 * CO:(k0 + nk) * CO],
                             start=True, stop=True)
            nc.vector.reduce_sum(
                out=partial[:, j, :],
                in_=ps_c.rearrange("p (k c) -> p c k", c=CO),
                axis=mybir.AxisListType.X)
        dsum = singles.tile([1, CO], F32)
        nc.vector.reduce_sum(out=dsum,
                             in_=partial.rearrange("p j c -> p c j"),
                             axis=mybir.AxisListType.X)
        eps_sb = singles.tile([1, 1], F32)
        nc.vector.memset(eps_sb, EPS)
        sstd = singles.tile([1, CO], F32)
        nc.scalar.activation(sstd, dsum, mybir.ActivationFunctionType.Sqrt,
                             bias=eps_sb[:, 0:1])
        demod = singles.tile([1, CO], F32)
        nc.vector.reciprocal(demod, sstd)

        # ---- broadcast demod across partitions -> [64, 64] ----
        ps_dm = wpsum.tile([64, CO], F32, tag="wps")
        nc.tensor.matmul(ps_dm, lhsT=ones1, rhs=demod, start=True, stop=True)
        dm_sb = singles.tile([64, CO], F32)
        nc.vector.tensor_copy(dm_sb, ps_dm)

    # ---- final scale + bf16 cast into WT ----
    # WT: plain weights duplicated on both partition halves (for singles).
    # WTs: lower half = W[koff], upper half = W[koff+1] — the stationary
    # operand of K=128 "pair" matmuls that compute two kw taps at once
    # against the +1-shifted upper x copy.
    WT = singles.tile([128, 27, CO], BF16)
    WTs = singles.tile([128, 27, CO], BF16)
    dm_view = dm_sb.unsqueeze(1).broadcast_to([64, 27, CO])
    nc.vector.tensor_mul(WT[0:64], acc.rearrange("p (k c) -> p k c", c=CO),
                         dm_view)
    nc.sync.dma_start(out=WT[64:128], in_=WT[0:64])
    nc.sync.dma_start(out=WTs[0:64], in_=WT[0:64])
    nc.sync.dma_start(out=WTs[64:128, 0:26], in_=WT[0:64, 1:27])
    nc.vector.memset(WTs[64:128, 26], 0.0)
    return WT, WTs


def _conv_offsets(d):
    """Valid (kd, kh, kw) taps for output d-plane d."""
    offs = []
    for kd in range(3):
        if 0 <= d + kd - 1 <= D - 1:
            for kh in range(3):
                for kw in range(3):
                    offs.append((kd, kh, kw))
    return offs




PLANE = (H + 2) * (W + 2)  # 1156, h/w zero-padded plane, flattened
ROWSPLIT = [(0, 11), (11, 11), (22, 10)]  # h-row tiles per d-plane


def _emit_conv(nc, tc, pools, aps, WT, WTs, xbf):
    """3x3x3 conv as shifted matmuls over flattened padded planes.

    HW constraints: moving operand = flat contiguous slice; one PSUM
    accumulation group must stay within one PE row-group footprint.

    Per output tile (nr h-rows of one d-plane, swept in padded-w space):
      - 9 "pair" matmuls, K=128: lower partitions stream x at tap
        (kd,kh,0), upper partitions hold x shifted by +1 element, so the
        same matmul accumulates tap (kd,kh,1). Full-row footprint.
      - 9 "single" matmuls, K=64 (tap (kd,kh,2)) on one row group.
    Pairs and singles accumulate in separate PSUM banks (group purity);
    the drain adds the two banks while stripping padded-w junk columns.
    Col groups run two tiles concurrently; singles use all 4 quadrants."""
    out_ap = aps["out"]
    osb_pool = pools["osb"]
    tiles = [(d, r0, nr) for d in range(D) for (r0, nr) in ROWSPLIT]
    with tc.tile_pool(name="cpsum", bufs=8, space="PSUM") as cpsum:
        for ti in range(0, len(tiles), 4):
            group = tiles[ti:ti + 4]
            ng = len(group)
            psp = [cpsum.tile([128, 512], F32, tag="cps", name=f"psp{j}")
                   for j in range(ng)]
            pss = [cpsum.tile([128, 512], F32, tag="cps", name=f"pss{j}")
                   for j in range(ng)]
            osbA = osb_pool.tile([128, 2, 11, W], F32, name="osbA")
            osbB = osb_pool.tile([128, 2, 11, W], F32, name="osbB")
            osbs = [osbA[0:64, 0], osbA[0:64, 1],
                    osbA[64:128, 0], osbA[64:128, 1]]
            del osbB  # reserved
            pair_l, sing_l = [], []
            for (d, r0, nr) in group:
                kds = [kd for kd in range(3) if 0 <= d + kd - 1 <= D - 1]
                pair_l.append([(kd, kh) for kd in kds for kh in range(3)])
                sing_l.append([(kd, kh) for kd in kds for kh in range(3)])
            nw = max(len(p) for p in pair_l)
            for i in range(nw):
                # pair wave: col groups alternate across the 4 tiles
                for j in range(ng):
                    if i >= len(pair_l[j]):
                        continue
                    d, r0, nr = group[j]
                    kd, kh = pair_l[j][i]
                    cp = (j % 2) * 64
                    koff = kd * 9 + kh * 3
                    n = nr * 34
                    off = 2 + (d + kd - 1) * PLANE + (r0 + kh) * 34 - 1
                    nc.tensor.matmul(
                        psp[j][cp:cp + 64, 0:n],
                        lhsT=WTs[:, koff, :],
                        rhs=xbf[0:128, off:off + n],
                        start=(i == 0), stop=(i == len(pair_l[j]) - 1))
                # single wave: 4 quadrants
                for j in range(ng):
                    if i >= len(sing_l[j]):
                        continue
                    d, r0, nr = group[j]
                    kd, kh = sing_l[j][i]
                    rg = (j // 2) * 64
                    cp = (j % 2) * 64
                    koff = kd * 9 + kh * 3 + 2
                    n = nr * 34
                    off = 2 + (d + kd - 1) * PLANE + (r0 + kh) * 34 + 1
                    if rg:
                        off -= 1
                    nc.tensor.matmul(
                        pss[j][cp:cp + 64, 0:n],
                        lhsT=WT[rg:rg + 64, koff, :],
                        rhs=xbf[rg:rg + 64, off:off + n],
                        start=(i == 0), stop=(i == len(sing_l[j]) - 1))
            # drain: add pair+single banks, strip junk cols, -> SBUF -> HBM
            for j, (d, r0, nr) in enumerate(group):
                cp = (j % 2) * 64
                pv = psp[j][cp:cp + 64, 0:nr * 34].rearrange(
                    "p (a b) -> p a b", b=34)[:, :, 1:W + 1]
                sv = pss[j][cp:cp + 64, 0:nr * 34].rearrange(
                    "p (a b) -> p a b", b=34)[:, :, 1:W + 1]
                ob = osbs[j][:, 0:nr, :]
                nc.scalar.copy(ob, pv)
                nc.vector.tensor_add(ob, ob, sv)
                nc.gpsimd.dma_start(out=out_ap[:, d, r0:r0 + nr, :], in_=ob)


def _build():
    nc = bacc.Bacc("TRN2", target_bir_lowering=False, debug=False)
    x = nc.dram_tensor("x", [CI, D, H, W], F32, kind="ExternalInput").ap()
    wk = nc.dram_tensor("wk", [128, BANK], F32, kind="ExternalInput").ap()
    fw = nc.dram_tensor("fw", [128, BANK, BANK], F32,
                        kind="ExternalInput").ap()
    fb = nc.dram_tensor("fb", [1, BANK], F32, kind="ExternalInput").ap()
    mwt = nc.dram_tensor("mwt", [128, BANK, CI], F32,
                         kind="ExternalInput").ap()
    mb = nc.dram_tensor("mb", [CI, 1], F32, kind="ExternalInput").ap()
    bankt = nc.dram_tensor("bankt", [CI, BANK, 27 * CO], F32,
                           kind="ExternalInput").ap()
    out = nc.dram_tensor("out", [CO, D, H, W], F32, kind="ExternalOutput").ap()
    aps = dict(x=x, wk=wk, fw=fw, fb=fb, mwt=mwt, mb=mb, bankt=bankt, out=out)

    with tile.TileContext(nc) as tc:
        with tc.tile_pool(name="singles", bufs=1) as singles, \
             tc.tile_pool(name="stg", bufs=2) as stg_pool, \
             tc.tile_pool(name="osb", bufs=4) as osb_pool:
            pools = dict(singles=singles, stg=stg_pool, osb=osb_pool)

            WT, WTs = _emit_weight_build(nc, tc, pools, aps)

            # x: f32 HBM -> zero-bordered SBUF staging (h/w padded) ->
            # bf16 flat planes, guards [2 front, 1 back]. Upper 64
            # partitions hold the copy shifted by +1 element (pair matmuls).
            xbf = singles.tile([128, 3 + D * PLANE], BF16)
            nc.vector.memset(xbf[:, 0:2], 0.0)
            nc.vector.memset(xbf[:, 2 + D * PLANE:3 + D * PLANE], 0.0)
            nc.vector.memset(xbf[64:128, 1 + D * PLANE:2 + D * PLANE], 0.0)
            stgs = [singles.tile([CI, DCH, H + 2, W + 2], F32, name=f"stg{i}")
                    for i in range(2)]
            for stg in stgs:
                nc.vector.memset(stg, 0.0)
            for s in range(D // DCH):
                stg = stgs[s % 2]
                for dd in range(DCH):
                    nc.sync.dma_start(out=stg[:, dd, 1:H + 1, 1:W + 1],
                                      in_=x[:, s * DCH + dd])
                lo, hi = 2 + s * DCH * PLANE, 2 + (s + 1) * DCH * PLANE
                nc.vector.tensor_copy(
                    xbf[0:64, lo:hi],
                    stg.rearrange("p a b c -> p (a b c)"))
                nc.scalar.dma_start(out=xbf[64:128, lo - 1:hi - 1],
                                    in_=xbf[0:64, lo:hi])

            _emit_conv(nc, tc, pools, aps, WT, WTs, xbf)

    nc.compile()
    return nc


def _shard_inputs(x, w, filter_w, filter_b, mod_w, mod_b, bank):
    """Host-side input marshalling: per-core shards + replicated params in
    the layouts the kernel expects."""
    fw_h = np.ascontiguousarray(
        filter_w.T.reshape(4, 128, BANK).transpose(1, 0, 2), np.float32)
    mwt_h = np.ascontiguousarray(
        mod_w.T.reshape(4, 128, CI).transpose(1, 0, 2), np.float32)
    bank_h = np.ascontiguousarray(
        bank.reshape(BANK, CO, CI, 27).transpose(2, 0, 3, 1)
        .reshape(CI, BANK, 27 * CO), np.float32)
    fb_h = np.ascontiguousarray(filter_b.reshape(1, BANK), np.float32)
    mb_h = np.ascontiguousarray(mod_b.reshape(CI, 1), np.float32)
    in_maps = []
    for i in range(NCORES):
        in_maps.append({
            "x": np.ascontiguousarray(x[i], np.float32),
            "wk": np.ascontiguousarray(w[i].reshape(4, 128).T, np.float32),
            "fw": fw_h, "fb": fb_h, "mwt": mwt_h, "mb": mb_h,
            "bankt": bank_h,
        })
    return in_maps


def _run(inputs, trace=False):
    if "nc" not in _CACHE:
        _CACHE["nc"] = _build()
    nc = _CACHE["nc"]
    in_maps = _shard_inputs(**inputs)
    res = bass_utils.run_bass_kernel_spmd(
        nc, in_maps, core_ids=list(range(NCORES)), trace=trace)
    out = np.stack([res.results[i]["out"] for i in range(NCORES)])
    return out.astype(np.float32), res


def kernel(**inputs):
    out, _ = _run(inputs, trace=False)
    return out


# revision 16
# speedup vs baseline: 1.1014x; 1.1014x over previous
"""AdaptiveModulatedConv3d — 8-core TRN2 Bass kernel.

Problem (hardcoded): BS=8, C_IN=C_OUT=64, K=3, STYLE_DIM=512, BANK=4,
D=H=W=32, pad=1, stride=1, f32 in/out.

Sharding: pure data-parallel over batch — each of the 8 NeuronCores gets one
sample, builds its per-sample demodulated conv weights on-device, and runs
its own 3D conv. No collectives.

Per-core conv strategy: the 3x3x3 conv is decomposed into 27 shifted
matmuls (contraction over C_IN=64) accumulating into PSUM. The PE 128x128
array is quadrant-packed: row-groups 0/64 hold two copies of x (bf16), so
two offset-matmuls run concurrently; col-groups 0/64 compute the two
h-halves of one output d-plane in the same PSUM bank. Boundary kernel taps
use narrowed-N matmuls instead of padding, so every DMA is contiguous.
"""

import numpy as np

import concourse.bass as bass
import concourse.tile as tile
from concourse import bacc, mybir
from concourse import bass_utils

F32 = mybir.dt.float32
BF16 = mybir.dt.bfloat16

BS = 8
CI = 64
CO = 64
KK = 3
SD = 512
BANK = 4
D = H = W = 32
EPS = 1e-8
NCORES = 8
DCH = 4  # d-planes per input-convert chunk

_CACHE = {}


def _emit_weight_build(nc, tc, pools, aps):
    """Build WT[128, 27, 64] bf16: WT[ci(+64), kd*9+kh*3+kw, co] =
    demodulated per-sample weight, duplicated on upper 64 partitions."""
    singles = pools["singles"]
    wk, fw, fb, mwt, mb, bankt = (
        aps["wk"], aps["fw"], aps["fb"], aps["mwt"], aps["mb"], aps["bankt"])

    # SBUF copies of the small params
    wk_sb = singles.tile([128, BANK], F32)
    nc.sync.dma_start(out=wk_sb, in_=wk)
    fw_sb = singles.tile([128, BANK, BANK], F32)
    nc.sync.dma_start(out=fw_sb, in_=fw)
    fb_sb = singles.tile([1, BANK], F32)
    nc.sync.dma_start(out=fb_sb, in_=fb)
    mwt_sb = singles.tile([128, BANK, CI], F32)
    nc.sync.dma_start(out=mwt_sb, in_=mwt)
    mb_sb = singles.tile([CI, 1], F32)
    nc.sync.dma_start(out=mb_sb, in_=mb)
    bank_sb = singles.tile([CI, BANK, 27 * CO], F32)
    nc.sync.dma_start(out=bank_sb, in_=bankt)

    ones1 = singles.tile([1, 64], F32)
    nc.vector.memset(ones1, 1.0)
    ones64 = singles.tile([64, 1], F32)
    nc.vector.memset(ones64, 1.0)

    with tc.tile_pool(name="wpsum", bufs=2, space="PSUM") as wpsum:
        # ---- filter weights: logits = w @ filter_w.T + filter_b ----
        ps_l = wpsum.tile([1, BANK], F32, tag="wps")
        for c in range(4):
            nc.tensor.matmul(ps_l, lhsT=wk_sb[:, c:c + 1], rhs=fw_sb[:, c, :],
                             start=(c == 0), stop=(c == 3))
        logits = singles.tile([1, BANK], F32)
        nc.vector.tensor_add(logits, ps_l, fb_sb)
        # softmax over 4 entries (logits are O(1), exp is safe)
        esb = singles.tile([1, BANK], F32)
        nc.scalar.activation(esb, logits, mybir.ActivationFunctionType.Exp)
        ssum = singles.tile([1, 1], F32)
        nc.vector.reduce_sum(out=ssum, in_=esb, axis=mybir.AxisListType.X)
        rsum = singles.tile([1, 1], F32)
        nc.vector.reciprocal(rsum, ssum)
        fwt = singles.tile([1, BANK], F32)
        nc.vector.tensor_scalar_mul(fwt, esb, rsum[:, 0:1])

        # ---- mod = w @ mod_w.T + mod_b  -> [ci, 1] ----
        ps_m = wpsum.tile([CI, 1], F32, tag="wps")
        for c in range(4):
            nc.tensor.matmul(ps_m, lhsT=mwt_sb[:, c, :], rhs=wk_sb[:, c:c + 1],
                             start=(c == 0), stop=(c == 3))
        mod_sb = singles.tile([CI, 1], F32)
        nc.vector.tensor_add(mod_sb, ps_m, mb_sb)

        # ---- broadcast fwt across partitions: [64, 4] ----
        ps_fb = wpsum.tile([64, BANK], F32, tag="wps")
        nc.tensor.matmul(ps_fb, lhsT=ones1, rhs=fwt, start=True, stop=True)
        fwt_b = singles.tile([64, BANK], F32)
        nc.vector.tensor_copy(fwt_b, ps_fb)

        # ---- weighted bank mix: acc[ci, koff*64+co] ----
        acc = singles.tile([CI, 27 * CO], F32)
        nc.vector.tensor_scalar_mul(acc, bank_sb[:, 0, :], fwt_b[:, 0:1])
        for n in range(1, 4):
            nc.vector.scalar_tensor_tensor(
                out=acc, in0=bank_sb[:, n, :], scalar=fwt_b[:, n:n + 1],
                in1=acc, op0=mybir.AluOpType.mult, op1=mybir.AluOpType.add)

        # ---- style modulation over input channels ----
        nc.vector.tensor_scalar_mul(acc, acc, mod_sb[:, 0:1])

        # ---- demod: rsqrt(sum_{ci,koff} mw^2 per co + eps) ----
        sq = singles.tile([CI, 27 * CO], F32)
        nc.scalar.square(sq, acc)
        partial = singles.tile([1, 4, CO], F32)
        chunks = [(0, 7), (7, 7), (14, 7), (21, 6)]  # koff ranges
        for j, (k0, nk) in enumerate(chunks):
            ps_c = wpsum.tile([1, nk * CO], F32, tag="wps")
            nc.tensor.matmul(ps_c, lhsT=ones64,
                             rhs=sq[:, k0 * CO:(k0 + nk) * CO],
                             start=True, stop=True)
            nc.vector.reduce_sum(
                out=partial[:, j, :],
                in_=ps_c.rearrange("p (k c) -> p c k", c=CO),
                axis=mybir.AxisListType.X)
        dsum = singles.tile([1, CO], F32)
        nc.vector.reduce_sum(out=dsum,
                             in_=partial.rearrange("p j c -> p c j"),
                             axis=mybir.AxisListType.X)
        eps_sb = singles.tile([1, 1], F32)
        nc.vector.memset(eps_sb, EPS)
        sstd = singles.tile([1, CO], F32)
        nc.scalar.activation(sstd, dsum, mybir.ActivationFunctionType.Sqrt,
                             bias=eps_sb[:, 0:1])
        demod = singles.tile([1, CO], F32)
        nc.vector.reciprocal(demod, sstd)

        # ---- broadcast demod across partitions -> [64, 64] ----
        ps_dm = wpsum.tile([64, CO], F32, tag="wps")
        nc.tensor.matmul(ps_dm, lhsT=ones1, rhs=demod, start=True, stop=True)
        dm_sb = singles.tile([64, CO], F32)
        nc.vector.tensor_copy(dm_sb, ps_dm)

    # ---- final scale + bf16 cast into WT ----
    # WT: plain weights duplicated on both partition halves (for singles).
    # WTs: lower half = W[koff], upper half = W[koff+1] — the stationary
    # operand of K=128 "pair" matmuls that compute two kw taps at once
    # against the +1-shifted upper x copy.
    WT = singles.tile([128, 27, CO], BF16)
    WTs = singles.tile([128, 27, CO], BF16)
    dm_view = dm_sb.unsqueeze(1).broadcast_to([64, 27, CO])
    nc.vector.tensor_mul(WT[0:64], acc.rearrange("p (k c) -> p k c", c=CO),
                         dm_view)
    nc.sync.dma_start(out=WT[64:128], in_=WT[0:64])
    nc.sync.dma_start(out=WTs[0:64], in_=WT[0:64])
    nc.sync.dma_start(out=WTs[64:128, 0:26], in_=WT[0:64, 1:27])
    nc.vector.memset(WTs[64:128, 26], 0.0)
    return WT, WTs


def _conv_offsets(d):
    """Valid (kd, kh, kw) taps for output d-plane d."""
    offs = []
    for kd in range(3):
        if 0 <= d + kd - 1 <= D - 1:
            for kh in range(3):
                for kw in range(3):
                    offs.append((kd, kh, kw))
    return offs




PLANE = (H + 2) * (W + 2)  # 1156, h/w zero-padded plane, flattened
ROWSPLIT = [(0, 11), (11, 11), (22, 10)]  # h-row tiles per d-plane


def _emit_conv(nc, tc, pools, aps, WT, WTs, xbf):
    """3x3x3 conv as 27 shifted matmuls per tile over flattened padded
    planes.

    HW constraints: moving operand = flat contiguous slice; one PSUM
    accumulation group must stay within ONE PE row group. Each of the 4 PE
    quadrants (row group x col group) owns an independent output tile in
    its own PSUM bank; the two x copies feed the two row groups (upper
    copy is stored shifted by +1 element, compensated in the offsets).
    Wave order groups the two rg0 matmuls then the two rg64 matmuls, so
    each LDWEIGHTS can pull ahead under the opposite row group's streams."""
    del WTs
    out_ap = aps["out"]
    osb_pool = pools["osb"]
    tiles = [(d, r0, nr) for d in range(D) for (r0, nr) in ROWSPLIT]
    quads = [(0, 0), (0, 64), (64, 0), (64, 64)]
    with tc.tile_pool(name="cpsum", bufs=8, space="PSUM") as cpsum:
        for ti in range(0, len(tiles), 4):
            group = tiles[ti:ti + 4]
            pss = [cpsum.tile([128, 512], F32, tag="cps", name=f"cps{j}")
                   for j in range(len(group))]
            osbA = osb_pool.tile([128, 2, 11, W], F32, name="osbA")
            osbs = [osbA[0:64, 0], osbA[64:128, 0],
                    osbA[0:64, 1], osbA[64:128, 1]]
            offs_l = [_conv_offsets(d) for (d, r0, nr) in group]
            nwaves = max(len(o) for o in offs_l)
            for i in range(nwaves):
                for j, (d, r0, nr) in enumerate(group):
                    offs = offs_l[j]
                    if i >= len(offs):
                        continue
                    kd, kh, kw = offs[i]
                    rg, cp = quads[j]
                    koff = kd * 9 + kh * 3 + kw
                    n = nr * 34
                    off = 2 + (d + kd - 1) * PLANE + (r0 + kh) * 34 + kw - 1
                    if rg:
                        off -= 1
                    nc.tensor.matmul(
                        pss[j][cp:cp + 64, 0:n],
                        lhsT=WT[rg:rg + 64, koff, :],
                        rhs=xbf[rg:rg + 64, off:off + n],
                        start=(i == 0), stop=(i == len(offs) - 1))
            # drain: strip padded-w junk columns, PSUM -> SBUF -> HBM
            for j, (d, r0, nr) in enumerate(group):
                cp = quads[j][1]
                pv = pss[j][cp:cp + 64, 0:nr * 34].rearrange(
                    "p (a b) -> p a b", b=34)[:, :, 1:W + 1]
                nc.scalar.copy(osbs[j][:, 0:nr, :], pv)
                nc.gpsimd.dma_start(out=out_ap[:, d, r0:r0 + nr, :],
                                    in_=osbs[j][:, 0:nr, :])


def _build():
    nc = bacc.Bacc("TRN2", target_bir_lowering=False, debug=False)
    x = nc.dram_tensor("x", [CI, D, H, W], F32, kind="ExternalInput").ap()
    wk = nc.dram_tensor("wk", [128, BANK], F32, kind="ExternalInput").ap()
    fw = nc.dram_tensor("fw", [128, BANK, BANK], F32,
                        kind="ExternalInput").ap()
    fb = nc.dram_tensor("fb", [1, BANK], F32, kind="ExternalInput").ap()
    mwt = nc.dram_tensor("mwt", [128, BANK, CI], F32,
                         kind="ExternalInput").ap()
    mb = nc.dram_tensor("mb", [CI, 1], F32, kind="ExternalInput").ap()
    bankt = nc.dram_tensor("bankt", [CI, BANK, 27 * CO], F32,
                           kind="ExternalInput").ap()
    out = nc.dram_tensor("out", [CO, D, H, W], F32, kind="ExternalOutput").ap()
    aps = dict(x=x, wk=wk, fw=fw, fb=fb, mwt=mwt, mb=mb, bankt=bankt, out=out)

    with tile.TileContext(nc) as tc:
        with tc.tile_pool(name="singles", bufs=1) as singles, \
             tc.tile_pool(name="stg", bufs=2) as stg_pool, \
             tc.tile_pool(name="osb", bufs=4) as osb_pool:
            pools = dict(singles=singles, stg=stg_pool, osb=osb_pool)

            WT, WTs = _emit_weight_build(nc, tc, pools, aps)

            # x: f32 HBM -> zero-bordered SBUF staging (h/w padded) ->
            # bf16 flat planes, guards [2 front, 1 back]. Upper 64
            # partitions hold the copy shifted by +1 element (pair matmuls).
            xbf = singles.tile([128, 3 + D * PLANE], BF16)
            nc.vector.memset(xbf[:, 0:2], 0.0)
            nc.vector.memset(xbf[:, 2 + D * PLANE:3 + D * PLANE], 0.0)
            nc.vector.memset(xbf[64:128, 1 + D * PLANE:2 + D * PLANE], 0.0)
            stgs = [singles.tile([CI, DCH, H + 2, W + 2], F32, name=f"stg{i}")
                    for i in range(2)]
            for stg in stgs:
                nc.vector.memset(stg, 0.0)
            for s in range(D // DCH):
                stg = stgs[s % 2]
                for dd in range(DCH):
                    nc.sync.dma_start(out=stg[:, dd, 1:H + 1, 1:W + 1],
                                      in_=x[:, s * DCH + dd])
                lo, hi = 2 + s * DCH * PLANE, 2 + (s + 1) * DCH * PLANE
                nc.vector.tensor_copy(
                    xbf[0:64, lo:hi],
                    stg.rearrange("p a b c -> p (a b c)"))
                nc.scalar.dma_start(out=xbf[64:128, lo - 1:hi - 1],
                                    in_=xbf[0:64, lo:hi])

            _emit_conv(nc, tc, pools, aps, WT, WTs, xbf)

    nc.compile()
    return nc


def _shard_inputs(x, w, filter_w, filter_b, mod_w, mod_b, bank):
    """Host-side input marshalling: per-core shards + replicated params in
    the layouts the kernel expects."""
    fw_h = np.ascontiguousarray(
        filter_w.T.reshape(4, 128, BANK).transpose(1, 0, 2), np.float32)
    mwt_h = np.ascontiguousarray(
        mod_w.T.reshape(4, 128, CI).transpose(1, 0, 2), np.float32)
    bank_h = np.ascontiguousarray(
        bank.reshape(BANK, CO, CI, 27).transpose(2, 0, 3, 1)
        .reshape(CI, BANK, 27 * CO), np.float32)
    fb_h = np.ascontiguousarray(filter_b.reshape(1, BANK), np.float32)
    mb_h = np.ascontiguousarray(mod_b.reshape(CI, 1), np.float32)
    in_maps = []
    for i in range(NCORES):
        in_maps.append({
            "x": np.ascontiguousarray(x[i], np.float32),
            "wk": np.ascontiguousarray(w[i].reshape(4, 128).T, np.float32),
            "fw": fw_h, "fb": fb_h, "mwt": mwt_h, "mb": mb_h,
            "bankt": bank_h,
        })
    return in_maps


def _run(inputs, trace=False):
    if "nc" not in _CACHE:
        _CACHE["nc"] = _build()
    nc = _CACHE["nc"]
    in_maps = _shard_inputs(**inputs)
    res = bass_utils.run_bass_kernel_spmd(
        nc, in_maps, core_ids=list(range(NCORES)), trace=trace)
    out = np.stack([res.results[i]["out"] for i in range(NCORES)])
    return out.astype(np.float32), res


def kernel(**inputs):
    out, _ = _run(inputs, trace=False)
    return out


# revision 17
# speedup vs baseline: 1.2950x; 1.1758x over previous
"""AdaptiveModulatedConv3d — 8-core TRN2 Bass kernel.

Problem (hardcoded): BS=8, C_IN=C_OUT=64, K=3, STYLE_DIM=512, BANK=4,
D=H=W=32, pad=1, stride=1, f32 in/out.

Sharding: pure data-parallel over batch — each of the 8 NeuronCores gets one
sample, builds its per-sample demodulated conv weights on-device, and runs
its own 3D conv. No collectives.

Per-core conv strategy: the 3x3x3 conv is decomposed into 27 shifted
matmuls (contraction over C_IN=64) accumulating into PSUM. The PE 128x128
array is quadrant-packed: row-groups 0/64 hold two copies of x (bf16), so
two offset-matmuls run concurrently; col-groups 0/64 compute the two
h-halves of one output d-plane in the same PSUM bank. Boundary kernel taps
use narrowed-N matmuls instead of padding, so every DMA is contiguous.
"""

import numpy as np

import concourse.bass as bass
import concourse.tile as tile
from concourse import bacc, mybir
from concourse import bass_utils

F32 = mybir.dt.float32
BF16 = mybir.dt.bfloat16

BS = 8
CI = 64
CO = 64
KK = 3
SD = 512
BANK = 4
D = H = W = 32
EPS = 1e-8
NCORES = 8
DCH = 4  # d-planes per input-convert chunk

_CACHE = {}


def _emit_weight_build(nc, tc, pools, aps):
    """Build WT[128, 27, 64] bf16: WT[ci(+64), kd*9+kh*3+kw, co] =
    demodulated per-sample weight, duplicated on upper 64 partitions."""
    singles = pools["singles"]
    wk, fw, fb, mwt, mb, bankt = (
        aps["wk"], aps["fw"], aps["fb"], aps["mwt"], aps["mb"], aps["bankt"])

    # SBUF copies of the small params
    wk_sb = singles.tile([128, BANK], F32)
    nc.sync.dma_start(out=wk_sb, in_=wk)
    fw_sb = singles.tile([128, BANK, BANK], F32)
    nc.sync.dma_start(out=fw_sb, in_=fw)
    fb_sb = singles.tile([1, BANK], F32)
    nc.sync.dma_start(out=fb_sb, in_=fb)
    mwt_sb = singles.tile([128, BANK, CI], F32)
    nc.sync.dma_start(out=mwt_sb, in_=mwt)
    mb_sb = singles.tile([CI, 1], F32)
    nc.sync.dma_start(out=mb_sb, in_=mb)
    bank_sb = singles.tile([CI, BANK, 27 * CO], F32)
    nc.sync.dma_start(out=bank_sb, in_=bankt)

    ones1 = singles.tile([1, 64], F32)
    nc.vector.memset(ones1, 1.0)
    ones64 = singles.tile([64, 1], F32)
    nc.vector.memset(ones64, 1.0)

    with tc.tile_pool(name="wpsum", bufs=2, space="PSUM") as wpsum:
        # ---- filter weights: logits = w @ filter_w.T + filter_b ----
        ps_l = wpsum.tile([1, BANK], F32, tag="wps")
        for c in range(4):
            nc.tensor.matmul(ps_l, lhsT=wk_sb[:, c:c + 1], rhs=fw_sb[:, c, :],
                             start=(c == 0), stop=(c == 3))
        logits = singles.tile([1, BANK], F32)
        nc.vector.tensor_add(logits, ps_l, fb_sb)
        # softmax over 4 entries (logits are O(1), exp is safe)
        esb = singles.tile([1, BANK], F32)
        nc.scalar.activation(esb, logits, mybir.ActivationFunctionType.Exp)
        ssum = singles.tile([1, 1], F32)
        nc.vector.reduce_sum(out=ssum, in_=esb, axis=mybir.AxisListType.X)
        rsum = singles.tile([1, 1], F32)
        nc.vector.reciprocal(rsum, ssum)
        fwt = singles.tile([1, BANK], F32)
        nc.vector.tensor_scalar_mul(fwt, esb, rsum[:, 0:1])

        # ---- mod = w @ mod_w.T + mod_b  -> [ci, 1] ----
        ps_m = wpsum.tile([CI, 1], F32, tag="wps")
        for c in range(4):
            nc.tensor.matmul(ps_m, lhsT=mwt_sb[:, c, :], rhs=wk_sb[:, c:c + 1],
                             start=(c == 0), stop=(c == 3))
        mod_sb = singles.tile([CI, 1], F32)
        nc.vector.tensor_add(mod_sb, ps_m, mb_sb)

        # ---- broadcast fwt across partitions: [64, 4] ----
        ps_fb = wpsum.tile([64, BANK], F32, tag="wps")
        nc.tensor.matmul(ps_fb, lhsT=ones1, rhs=fwt, start=True, stop=True)
        fwt_b = singles.tile([64, BANK], F32)
        nc.vector.tensor_copy(fwt_b, ps_fb)

        # ---- weighted bank mix: acc[ci, koff*64+co] ----
        acc = singles.tile([CI, 27 * CO], F32)
        nc.vector.tensor_scalar_mul(acc, bank_sb[:, 0, :], fwt_b[:, 0:1])
        for n in range(1, 4):
            nc.vector.scalar_tensor_tensor(
                out=acc, in0=bank_sb[:, n, :], scalar=fwt_b[:, n:n + 1],
                in1=acc, op0=mybir.AluOpType.mult, op1=mybir.AluOpType.add)

        # ---- style modulation over input channels ----
        nc.vector.tensor_scalar_mul(acc, acc, mod_sb[:, 0:1])

        # ---- demod: rsqrt(sum_{ci,koff} mw^2 per co + eps) ----
        sq = singles.tile([CI, 27 * CO], F32)
        nc.scalar.square(sq, acc)
        partial = singles.tile([1, 4, CO], F32)
        chunks = [(0, 7), (7, 7), (14, 7), (21, 6)]  # koff ranges
        for j, (k0, nk) in enumerate(chunks):
            ps_c = wpsum.tile([1, nk * CO], F32, tag="wps")
            nc.tensor.matmul(ps_c, lhsT=ones64,
                             rhs=sq[:, k0 * CO:(k0 + nk) * CO],
                             start=True, stop=True)
            nc.vector.reduce_sum(
                out=partial[:, j, :],
                in_=ps_c.rearrange("p (k c) -> p c k", c=CO),
                axis=mybir.AxisListType.X)
        dsum = singles.tile([1, CO], F32)
        nc.vector.reduce_sum(out=dsum,
                             in_=partial.rearrange("p j c -> p c j"),
                             axis=mybir.AxisListType.X)
        eps_sb = singles.tile([1, 1], F32)
        nc.vector.memset(eps_sb, EPS)
        sstd = singles.tile([1, CO], F32)
        nc.scalar.activation(sstd, dsum, mybir.ActivationFunctionType.Sqrt,
                             bias=eps_sb[:, 0:1])
        demod = singles.tile([1, CO], F32)
        nc.vector.reciprocal(demod, sstd)

        # ---- broadcast demod across partitions -> [64, 64] ----
        ps_dm = wpsum.tile([64, CO], F32, tag="wps")
        nc.tensor.matmul(ps_dm, lhsT=ones1, rhs=demod, start=True, stop=True)
        dm_sb = singles.tile([64, CO], F32)
        nc.vector.tensor_copy(dm_sb, ps_dm)

    # ---- final scale + bf16 cast into WT ----
    # WT: plain weights duplicated on both partition halves (for singles).
    # WTs: lower half = W[koff], upper half = W[koff+1] — the stationary
    # operand of K=128 "pair" matmuls that compute two kw taps at once
    # against the +1-shifted upper x copy.
    WT = singles.tile([128, 27, CO], BF16)
    WTs = singles.tile([128, 27, CO], BF16)
    dm_view = dm_sb.unsqueeze(1).broadcast_to([64, 27, CO])
    nc.vector.tensor_mul(WT[0:64], acc.rearrange("p (k c) -> p k c", c=CO),
                         dm_view)
    nc.sync.dma_start(out=WT[64:128], in_=WT[0:64])
    nc.sync.dma_start(out=WTs[0:64], in_=WT[0:64])
    nc.sync.dma_start(out=WTs[64:128, 0:26], in_=WT[0:64, 1:27])
    nc.vector.memset(WTs[64:128, 26], 0.0)
    return WT, WTs


def _conv_offsets(d):
    """Valid (kd, kh, kw) taps for output d-plane d."""
    offs = []
    for kd in range(3):
        if 0 <= d + kd - 1 <= D - 1:
            for kh in range(3):
                for kw in range(3):
                    offs.append((kd, kh, kw))
    return offs




PLANE = (H + 2) * (W + 2)  # 1156, h/w zero-padded plane, flattened
ROWSPLIT = [(0, 11), (11, 11), (22, 10)]  # h-row tiles per d-plane


def _emit_conv(nc, tc, pools, aps, WT, WTs, xbf):
    """3x3x3 conv as 27 shifted matmuls per tile over flattened padded
    planes.

    HW constraints: moving operand = flat contiguous slice; one PSUM
    accumulation group must stay within ONE PE row group. Each of the 4 PE
    quadrants (row group x col group) owns an independent output tile in
    its own PSUM bank; the two x copies feed the two row groups (upper
    copy is stored shifted by +1 element, compensated in the offsets).
    Wave order groups the two rg0 matmuls then the two rg64 matmuls, so
    each LDWEIGHTS can pull ahead under the opposite row group's streams."""
    del WTs
    out_ap = aps["out"]
    osb_pool = pools["osb"]
    tiles = [(d, r0, nr) for d in range(D) for (r0, nr) in ROWSPLIT]
    quads = [(0, 0), (64, 0), (0, 64), (64, 64)]
    with tc.tile_pool(name="cpsum", bufs=8, space="PSUM") as cpsum:
        for ti in range(0, len(tiles), 4):
            group = tiles[ti:ti + 4]
            pss = [cpsum.tile([128, 512], F32, tag="cps", name=f"cps{j}")
                   for j in range(len(group))]
            osbA = osb_pool.tile([128, 2, 11, W], F32, name="osbA")
            osbs = [osbA[0:64, 0], osbA[0:64, 1],
                    osbA[64:128, 0], osbA[64:128, 1]]
            offs_l = [_conv_offsets(d) for (d, r0, nr) in group]
            nwaves = max(len(o) for o in offs_l)
            for i in range(nwaves):
                for j, (d, r0, nr) in enumerate(group):
                    offs = offs_l[j]
                    if i >= len(offs):
                        continue
                    kd, kh, kw = offs[i]
                    rg, cp = quads[j]
                    koff = kd * 9 + kh * 3 + kw
                    n = nr * 34
                    off = 2 + (d + kd - 1) * PLANE + (r0 + kh) * 34 + kw - 1
                    if rg:
                        off -= 1
                    nc.tensor.matmul(
                        pss[j][cp:cp + 64, 0:n],
                        lhsT=WT[rg:rg + 64, koff, :],
                        rhs=xbf[rg:rg + 64, off:off + n],
                        start=(i == 0), stop=(i == len(offs) - 1))
            # drain: strip padded-w junk columns, PSUM -> SBUF -> HBM
            for j, (d, r0, nr) in enumerate(group):
                cp = quads[j][1]
                pv = pss[j][cp:cp + 64, 0:nr * 34].rearrange(
                    "p (a b) -> p a b", b=34)[:, :, 1:W + 1]
                nc.scalar.copy(osbs[j][:, 0:nr, :], pv)
                nc.gpsimd.dma_start(out=out_ap[:, d, r0:r0 + nr, :],
                                    in_=osbs[j][:, 0:nr, :])


def _build():
    nc = bacc.Bacc("TRN2", target_bir_lowering=False, debug=False)
    x = nc.dram_tensor("x", [CI, D, H, W], F32, kind="ExternalInput").ap()
    wk = nc.dram_tensor("wk", [128, BANK], F32, kind="ExternalInput").ap()
    fw = nc.dram_tensor("fw", [128, BANK, BANK], F32,
                        kind="ExternalInput").ap()
    fb = nc.dram_tensor("fb", [1, BANK], F32, kind="ExternalInput").ap()
    mwt = nc.dram_tensor("mwt", [128, BANK, CI], F32,
                         kind="ExternalInput").ap()
    mb = nc.dram_tensor("mb", [CI, 1], F32, kind="ExternalInput").ap()
    bankt = nc.dram_tensor("bankt", [CI, BANK, 27 * CO], F32,
                           kind="ExternalInput").ap()
    out = nc.dram_tensor("out", [CO, D, H, W], F32, kind="ExternalOutput").ap()
    aps = dict(x=x, wk=wk, fw=fw, fb=fb, mwt=mwt, mb=mb, bankt=bankt, out=out)

    with tile.TileContext(nc) as tc:
        with tc.tile_pool(name="singles", bufs=1) as singles, \
             tc.tile_pool(name="stg", bufs=2) as stg_pool, \
             tc.tile_pool(name="osb", bufs=4) as osb_pool:
            pools = dict(singles=singles, stg=stg_pool, osb=osb_pool)

            WT, WTs = _emit_weight_build(nc, tc, pools, aps)

            # x: f32 HBM -> zero-bordered SBUF staging (h/w padded) ->
            # bf16 flat planes, guards [2 front, 1 back]. Upper 64
            # partitions hold the copy shifted by +1 element (pair matmuls).
            xbf = singles.tile([128, 3 + D * PLANE], BF16)
            nc.vector.memset(xbf[:, 0:2], 0.0)
            nc.vector.memset(xbf[:, 2 + D * PLANE:3 + D * PLANE], 0.0)
            nc.vector.memset(xbf[64:128, 1 + D * PLANE:2 + D * PLANE], 0.0)
            stgs = [singles.tile([CI, DCH, H + 2, W + 2], F32, name=f"stg{i}")
                    for i in range(2)]
            for stg in stgs:
                nc.vector.memset(stg, 0.0)
            for s in range(D // DCH):
                stg = stgs[s % 2]
                for dd in range(DCH):
                    nc.sync.dma_start(out=stg[:, dd, 1:H + 1, 1:W + 1],
                                      in_=x[:, s * DCH + dd])
                lo, hi = 2 + s * DCH * PLANE, 2 + (s + 1) * DCH * PLANE
                nc.vector.tensor_copy(
                    xbf[0:64, lo:hi],
                    stg.rearrange("p a b c -> p (a b c)"))
                nc.scalar.dma_start(out=xbf[64:128, lo - 1:hi - 1],
                                    in_=xbf[0:64, lo:hi])

            _emit_conv(nc, tc, pools, aps, WT, WTs, xbf)

    nc.compile()
    return nc


def _shard_inputs(x, w, filter_w, filter_b, mod_w, mod_b, bank):
    """Host-side input marshalling: per-core shards + replicated params in
    the layouts the kernel expects."""
    fw_h = np.ascontiguousarray(
        filter_w.T.reshape(4, 128, BANK).transpose(1, 0, 2), np.float32)
    mwt_h = np.ascontiguousarray(
        mod_w.T.reshape(4, 128, CI).transpose(1, 0, 2), np.float32)
    bank_h = np.ascontiguousarray(
        bank.reshape(BANK, CO, CI, 27).transpose(2, 0, 3, 1)
        .reshape(CI, BANK, 27 * CO), np.float32)
    fb_h = np.ascontiguousarray(filter_b.reshape(1, BANK), np.float32)
    mb_h = np.ascontiguousarray(mod_b.reshape(CI, 1), np.float32)
    in_maps = []
    for i in range(NCORES):
        in_maps.append({
            "x": np.ascontiguousarray(x[i], np.float32),
            "wk": np.ascontiguousarray(w[i].reshape(4, 128).T, np.float32),
            "fw": fw_h, "fb": fb_h, "mwt": mwt_h, "mb": mb_h,
            "bankt": bank_h,
        })
    return in_maps


def _run(inputs, trace=False):
    if "nc" not in _CACHE:
        _CACHE["nc"] = _build()
    nc = _CACHE["nc"]
    in_maps = _shard_inputs(**inputs)
    res = bass_utils.run_bass_kernel_spmd(
        nc, in_maps, core_ids=list(range(NCORES)), trace=trace)
    out = np.stack([res.results[i]["out"] for i in range(NCORES)])
    return out.astype(np.float32), res


def kernel(**inputs):
    out, _ = _run(inputs, trace=False)
    return out


# revision 18
# speedup vs baseline: 1.5025x; 1.1602x over previous
"""AdaptiveModulatedConv3d — 8-core TRN2 Bass kernel.

Problem (hardcoded): BS=8, C_IN=C_OUT=64, K=3, STYLE_DIM=512, BANK=4,
D=H=W=32, pad=1, stride=1, f32 in/out.

Sharding: pure data-parallel over batch — each of the 8 NeuronCores gets one
sample, builds its per-sample demodulated conv weights on-device, and runs
its own 3D conv. No collectives.

Per-core conv strategy: the 3x3x3 conv is decomposed into 27 shifted
matmuls (contraction over C_IN=64) accumulating into PSUM. The PE 128x128
array is quadrant-packed: row-groups 0/64 hold two copies of x (bf16), so
two offset-matmuls run concurrently; col-groups 0/64 compute the two
h-halves of one output d-plane in the same PSUM bank. Boundary kernel taps
use narrowed-N matmuls instead of padding, so every DMA is contiguous.
"""

import numpy as np

import concourse.bass as bass
import concourse.tile as tile
from concourse import bacc, mybir
from concourse import bass_utils

F32 = mybir.dt.float32
BF16 = mybir.dt.bfloat16

BS = 8
CI = 64
CO = 64
KK = 3
SD = 512
BANK = 4
D = H = W = 32
EPS = 1e-8
NCORES = 8
DCH = 4  # d-planes per input-convert chunk

_CACHE = {}


def _emit_weight_build(nc, tc, pools, aps):
    """Build WT[128, 27, 64] bf16: WT[ci(+64), kd*9+kh*3+kw, co] =
    demodulated per-sample weight, duplicated on upper 64 partitions."""
    singles = pools["singles"]
    wk, fw, fb, mwt, mb, bankt = (
        aps["wk"], aps["fw"], aps["fb"], aps["mwt"], aps["mb"], aps["bankt"])

    # SBUF copies of the small params
    wk_sb = singles.tile([128, BANK], F32)
    nc.sync.dma_start(out=wk_sb, in_=wk)
    fw_sb = singles.tile([128, BANK, BANK], F32)
    nc.sync.dma_start(out=fw_sb, in_=fw)
    fb_sb = singles.tile([1, BANK], F32)
    nc.sync.dma_start(out=fb_sb, in_=fb)
    mwt_sb = singles.tile([128, BANK, CI], F32)
    nc.sync.dma_start(out=mwt_sb, in_=mwt)
    mb_sb = singles.tile([CI, 1], F32)
    nc.sync.dma_start(out=mb_sb, in_=mb)
    bank_sb = singles.tile([CI, BANK, 27 * CO], F32)
    nc.sync.dma_start(out=bank_sb, in_=bankt)

    ones1 = singles.tile([1, 64], F32)
    nc.vector.memset(ones1, 1.0)
    ones64 = singles.tile([64, 1], F32)
    nc.vector.memset(ones64, 1.0)

    with tc.tile_pool(name="wpsum", bufs=2, space="PSUM") as wpsum:
        # ---- filter weights: logits = w @ filter_w.T + filter_b ----
        ps_l = wpsum.tile([1, BANK], F32, tag="wps")
        for c in range(4):
            nc.tensor.matmul(ps_l, lhsT=wk_sb[:, c:c + 1], rhs=fw_sb[:, c, :],
                             start=(c == 0), stop=(c == 3))
        logits = singles.tile([1, BANK], F32)
        nc.vector.tensor_add(logits, ps_l, fb_sb)
        # softmax over 4 entries (logits are O(1), exp is safe)
        esb = singles.tile([1, BANK], F32)
        nc.scalar.activation(esb, logits, mybir.ActivationFunctionType.Exp)
        ssum = singles.tile([1, 1], F32)
        nc.vector.reduce_sum(out=ssum, in_=esb, axis=mybir.AxisListType.X)
        rsum = singles.tile([1, 1], F32)
        nc.vector.reciprocal(rsum, ssum)
        fwt = singles.tile([1, BANK], F32)
        nc.vector.tensor_scalar_mul(fwt, esb, rsum[:, 0:1])

        # ---- mod = w @ mod_w.T + mod_b  -> [ci, 1] ----
        ps_m = wpsum.tile([CI, 1], F32, tag="wps")
        for c in range(4):
            nc.tensor.matmul(ps_m, lhsT=mwt_sb[:, c, :], rhs=wk_sb[:, c:c + 1],
                             start=(c == 0), stop=(c == 3))
        mod_sb = singles.tile([CI, 1], F32)
        nc.vector.tensor_add(mod_sb, ps_m, mb_sb)

        # ---- broadcast fwt across partitions: [64, 4] ----
        ps_fb = wpsum.tile([64, BANK], F32, tag="wps")
        nc.tensor.matmul(ps_fb, lhsT=ones1, rhs=fwt, start=True, stop=True)
        fwt_b = singles.tile([64, BANK], F32)
        nc.vector.tensor_copy(fwt_b, ps_fb)

        # ---- weighted bank mix: acc[ci, koff*64+co] ----
        acc = singles.tile([CI, 27 * CO], F32)
        nc.vector.tensor_scalar_mul(acc, bank_sb[:, 0, :], fwt_b[:, 0:1])
        for n in range(1, 4):
            nc.vector.scalar_tensor_tensor(
                out=acc, in0=bank_sb[:, n, :], scalar=fwt_b[:, n:n + 1],
                in1=acc, op0=mybir.AluOpType.mult, op1=mybir.AluOpType.add)

        # ---- style modulation over input channels ----
        nc.vector.tensor_scalar_mul(acc, acc, mod_sb[:, 0:1])

        # ---- demod: rsqrt(sum_{ci,koff} mw^2 per co + eps) ----
        sq = singles.tile([CI, 27 * CO], F32)
        nc.scalar.square(sq, acc)
        partial = singles.tile([1, 4, CO], F32)
        chunks = [(0, 7), (7, 7), (14, 7), (21, 6)]  # koff ranges
        for j, (k0, nk) in enumerate(chunks):
            ps_c = wpsum.tile([1, nk * CO], F32, tag="wps")
            nc.tensor.matmul(ps_c, lhsT=ones64,
                             rhs=sq[:, k0 * CO:(k0 + nk) * CO],
                             start=True, stop=True)
            nc.vector.reduce_sum(
                out=partial[:, j, :],
                in_=ps_c.rearrange("p (k c) -> p c k", c=CO),
                axis=mybir.AxisListType.X)
        dsum = singles.tile([1, CO], F32)
        nc.vector.reduce_sum(out=dsum,
                             in_=partial.rearrange("p j c -> p c j"),
                             axis=mybir.AxisListType.X)
        eps_sb = singles.tile([1, 1], F32)
        nc.vector.memset(eps_sb, EPS)
        sstd = singles.tile([1, CO], F32)
        nc.scalar.activation(sstd, dsum, mybir.ActivationFunctionType.Sqrt,
                             bias=eps_sb[:, 0:1])
        demod = singles.tile([1, CO], F32)
        nc.vector.reciprocal(demod, sstd)

        # ---- broadcast demod across partitions -> [64, 64] ----
        ps_dm = wpsum.tile([64, CO], F32, tag="wps")
        nc.tensor.matmul(ps_dm, lhsT=ones1, rhs=demod, start=True, stop=True)
        dm_sb = singles.tile([64, CO], F32)
        nc.vector.tensor_copy(dm_sb, ps_dm)

    # ---- final scale + bf16 cast into WT ----
    # WT: plain weights duplicated on both partition halves (for singles).
    # WTs: lower half = W[koff], upper half = W[koff+1] — the stationary
    # operand of K=128 "pair" matmuls that compute two kw taps at once
    # against the +1-shifted upper x copy.
    WT = singles.tile([128, 27, CO], BF16)
    WTs = singles.tile([128, 27, CO], BF16)
    dm_view = dm_sb.unsqueeze(1).broadcast_to([64, 27, CO])
    nc.vector.tensor_mul(WT[0:64], acc.rearrange("p (k c) -> p k c", c=CO),
                         dm_view)
    nc.sync.dma_start(out=WT[64:128], in_=WT[0:64])
    nc.sync.dma_start(out=WTs[0:64], in_=WT[0:64])
    nc.sync.dma_start(out=WTs[64:128, 0:26], in_=WT[0:64, 1:27])
    nc.vector.memset(WTs[64:128, 26], 0.0)
    return WT, WTs


def _conv_offsets(d):
    """Valid (kd, kh, kw) taps for output d-plane d."""
    offs = []
    for kd in range(3):
        if 0 <= d + kd - 1 <= D - 1:
            for kh in range(3):
                for kw in range(3):
                    offs.append((kd, kh, kw))
    return offs




PLANE = (H + 2) * (W + 2)  # 1156, h/w zero-padded plane, flattened
ROWSPLIT = [(0, 11), (11, 11), (22, 10)]  # h-row tiles per d-plane


def _emit_conv(nc, tc, pools, aps, WT, WTs, xbf):
    """3x3x3 conv as 27 shifted matmuls per tile over flattened padded
    planes.

    HW constraints: moving operand = flat contiguous slice; one PSUM
    accumulation group must stay within ONE PE row group. Each of the 4 PE
    quadrants (row group x col group) owns an independent output tile in
    its own PSUM bank; the two x copies feed the two row groups (upper
    copy is stored shifted by +1 element, compensated in the offsets).
    Wave order groups the two rg0 matmuls then the two rg64 matmuls, so
    each LDWEIGHTS can pull ahead under the opposite row group's streams."""
    del WTs
    out_ap = aps["out"]
    osb_pool = pools["osb"]
    tiles = [(d, r0, nr) for d in range(D) for (r0, nr) in ROWSPLIT]
    quads = [(0, 0), (64, 0), (0, 64), (64, 64)]
    with tc.tile_pool(name="cpsum", bufs=8, space="PSUM") as cpsum:
        for ti in range(0, len(tiles), 4):
            group = tiles[ti:ti + 4]
            pss = [cpsum.tile([128, 512], F32, tag="cps", name=f"cps{j}")
                   for j in range(len(group))]
            osbA = osb_pool.tile([128, 2, 11, W], F32, name="osbA")
            osbs = [osbA[0:64, 0], osbA[0:64, 1],
                    osbA[64:128, 0], osbA[64:128, 1]]
            offs_l = [_conv_offsets(d) for (d, r0, nr) in group]
            nwaves = max(len(o) for o in offs_l)
            for i in range(nwaves):
                for j, (d, r0, nr) in enumerate(group):
                    offs = offs_l[j]
                    if i >= len(offs):
                        continue
                    kd, kh, kw = offs[i]
                    rg, cp = quads[j]
                    koff = kd * 9 + kh * 3 + kw
                    n = nr * 34
                    off = 2 + (d + kd - 1) * PLANE + (r0 + kh) * 34 + kw - 1
                    if rg:
                        off -= 1
                    nc.tensor.matmul(
                        pss[j][cp:cp + 64, 0:n],
                        lhsT=WT[rg:rg + 64, koff, :],
                        rhs=xbf[rg:rg + 64, off:off + n],
                        start=(i == 0), stop=(i == len(offs) - 1))
            # drain: strip padded-w junk columns, PSUM -> SBUF -> HBM
            for j, (d, r0, nr) in enumerate(group):
                cp = quads[j][1]
                pv = pss[j][cp:cp + 64, 0:nr * 34].rearrange(
                    "p (a b) -> p a b", b=34)[:, :, 1:W + 1]
                nc.scalar.copy(osbs[j][:, 0:nr, :], pv)
                eng = nc.gpsimd if j % 2 == 0 else nc.sync
                eng.dma_start(out=out_ap[:, d, r0:r0 + nr, :],
                              in_=osbs[j][:, 0:nr, :])


def _build():
    nc = bacc.Bacc("TRN2", target_bir_lowering=False, debug=False)
    x = nc.dram_tensor("x", [CI, D, H, W], F32, kind="ExternalInput").ap()
    wk = nc.dram_tensor("wk", [128, BANK], F32, kind="ExternalInput").ap()
    fw = nc.dram_tensor("fw", [128, BANK, BANK], F32,
                        kind="ExternalInput").ap()
    fb = nc.dram_tensor("fb", [1, BANK], F32, kind="ExternalInput").ap()
    mwt = nc.dram_tensor("mwt", [128, BANK, CI], F32,
                         kind="ExternalInput").ap()
    mb = nc.dram_tensor("mb", [CI, 1], F32, kind="ExternalInput").ap()
    bankt = nc.dram_tensor("bankt", [CI, BANK, 27 * CO], F32,
                           kind="ExternalInput").ap()
    out = nc.dram_tensor("out", [CO, D, H, W], F32, kind="ExternalOutput").ap()
    aps = dict(x=x, wk=wk, fw=fw, fb=fb, mwt=mwt, mb=mb, bankt=bankt, out=out)

    with tile.TileContext(nc) as tc:
        with tc.tile_pool(name="singles", bufs=1) as singles, \
             tc.tile_pool(name="stg", bufs=2) as stg_pool, \
             tc.tile_pool(name="osb", bufs=4) as osb_pool:
            pools = dict(singles=singles, stg=stg_pool, osb=osb_pool)

            WT, WTs = _emit_weight_build(nc, tc, pools, aps)

            # x: f32 HBM -> flat SBUF staging (contiguous DMA) -> bf16
            # cast with strided dest into padded planes. Borders zeroed
            # once by strided memsets. Upper 64 partitions hold the copy
            # shifted by +1 element (offsets compensate).
            xbf = singles.tile([128, 3 + D * PLANE], BF16)
            xv = xbf.rearrange("p (g e) -> p g e", g=1)  # keep AP helper
            del xv
            nc.vector.memset(xbf[:, 0:2], 0.0)
            nc.vector.memset(xbf[:, 2 + D * PLANE:3 + D * PLANE], 0.0)
            nc.vector.memset(xbf[64:128, 1 + D * PLANE:2 + D * PLANE], 0.0)
            pl_all = xbf[:, 2:2 + D * PLANE].rearrange(
                "p (d h w) -> p d h w", h=H + 2, w=W + 2)
            nc.vector.memset(pl_all[:, :, 0, :], 0.0)
            nc.vector.memset(pl_all[:, :, H + 1, :], 0.0)
            nc.vector.memset(pl_all[:, :, :, 0], 0.0)
            nc.vector.memset(pl_all[:, :, :, W + 1], 0.0)
            for s in range(D // DCH):
                stg = stg_pool.tile([CI, DCH, H, W], F32)
                nc.sync.dma_start(out=stg, in_=x[:, s * DCH:(s + 1) * DCH])
                for dd in range(DCH):
                    p = s * DCH + dd
                    b2 = 2 + p * PLANE + (W + 2) + 1
                    dst = xbf[0:64, b2:b2 + H * (W + 2)].rearrange(
                        "p (h w) -> p h w", w=W + 2)[:, :, 0:W]
                    nc.vector.tensor_copy(dst, stg[:, dd])
                lo, hi = 2 + s * DCH * PLANE, 2 + (s + 1) * DCH * PLANE
                nc.scalar.dma_start(out=xbf[64:128, lo - 1:hi - 1],
                                    in_=xbf[0:64, lo:hi])

            _emit_conv(nc, tc, pools, aps, WT, WTs, xbf)

    nc.compile()
    return nc


def _shard_inputs(x, w, filter_w, filter_b, mod_w, mod_b, bank):
    """Host-side input marshalling: per-core shards + replicated params in
    the layouts the kernel expects."""
    fw_h = np.ascontiguousarray(
        filter_w.T.reshape(4, 128, BANK).transpose(1, 0, 2), np.float32)
    mwt_h = np.ascontiguousarray(
        mod_w.T.reshape(4, 128, CI).transpose(1, 0, 2), np.float32)
    bank_h = np.ascontiguousarray(
        bank.reshape(BANK, CO, CI, 27).transpose(2, 0, 3, 1)
        .reshape(CI, BANK, 27 * CO), np.float32)
    fb_h = np.ascontiguousarray(filter_b.reshape(1, BANK), np.float32)
    mb_h = np.ascontiguousarray(mod_b.reshape(CI, 1), np.float32)
    in_maps = []
    for i in range(NCORES):
        in_maps.append({
            "x": np.ascontiguousarray(x[i], np.float32),
            "wk": np.ascontiguousarray(w[i].reshape(4, 128).T, np.float32),
            "fw": fw_h, "fb": fb_h, "mwt": mwt_h, "mb": mb_h,
            "bankt": bank_h,
        })
    return in_maps


def _run(inputs, trace=False):
    if "nc" not in _CACHE:
        _CACHE["nc"] = _build()
    nc = _CACHE["nc"]
    in_maps = _shard_inputs(**inputs)
    res = bass_utils.run_bass_kernel_spmd(
        nc, in_maps, core_ids=list(range(NCORES)), trace=trace)
    out = np.stack([res.results[i]["out"] for i in range(NCORES)])
    return out.astype(np.float32), res


def kernel(**inputs):
    out, _ = _run(inputs, trace=False)
    return out
